# revision 9
# baseline (speedup 1.0000x reference)
"""Trainium2 Bass kernel for 24-rotation (octahedral) 3D conv (ConvZ3P24).

Problem: x (2,4,64,64,64) f32, weight (8,4,3,3,3), bias (8,)
  -> y (2,24,8,64,64,64):  conv3d(x, rotated_filter_bank) + bias,
  stride 1, pad 1, 24 proper octahedral rotations x 8 cout = 192 channels.

Sharding: 8 cores = batch(2) x depth-chunks(4 x 16 planes). Each core
computes all 192 channels for its 16 output planes.

v2 design (vs v1 baseline at ~220us):
  - y written as bf16 (host upcasts) -> output HBM traffic halved (25.2MB).
  - input is NOT pre-im2col'd to 108 rows; host preps xs9[18, 36, 4224]:
    for each padded input plane p, the 9 (kh,kw)-shifted copies of the 4
    cin planes (rows r=(kh,kw,ci)).  5.5MB/core instead of 14.6MB.
  - device keeps two half-plane tiles EXP_L/EXP_R [108, 2112].  Rows are
    3 slots of 36; slot s holds plane p (p%3==s).  Each input plane is
    DMA'd in once and reused by the 3 output planes that need it; the
    kd->slot mapping is absorbed by 3 host-permuted lhsT variants
    w3[r], r=d%3 (slot s of variant r holds taps for kd=(s-r)%3).
  - per output plane d: 4 quads (mh, L/R) x 2 psum-pairs: 16 matmuls
    [108,96]^T @ [108,512] -> PSUM.  Evac in [96,1024] groups (2 banks),
    fused bias add + bf16 cast, alternating DVE/ACT, into per-(d,mh)
    stage tiles [96,4096]; one 768KB out-DMA per (d,mh).
  - HBM/iter ~= 25.2 (out) + 5.5 (in) + 0.1 = 30.8MB -> ~86us floor.
"""

import itertools
from contextlib import ExitStack

import numpy as np

# ---------------------------------------------------------------- constants
CIN = 4
COUT = 8
N_ROT = 24
KS = 3
DHW = 64
PH = 66            # padded plane side
JW = 4096          # xs9 row length (de-padded: 64 rows x 64 cols)
HCOLS = 2048       # half-plane tile cols (32 h-rows * 64, contiguous)
N_CORES = 8
DCHUNK = 16        # output planes per core
SLAB_D = 18        # input padded planes per core (16 + 2 halo)
M = 192            # total output channels (24 rot * 8 cout)
MHALF = 96
NROW = 108         # contraction rows = 27 taps * 4 cin
NCOL = 512         # matmul free dim (8 h-rows * 64)

_CACHE = {}


def _rot_index_maps(k):
    """Source voxel indices (d,h,w) for the 24 proper octahedral rotations."""
    m = (k - 1) // 2
    mats = []
    for perm in itertools.permutations(range(3)):
        for signs in itertools.product([1, -1], repeat=3):
            R = np.zeros((3, 3))
            for i in range(3):
                R[i, perm[i]] = signs[i]
            if np.linalg.det(R) > 0.5:
                mats.append(R)
    c = np.arange(k) - m
    cz, cy, cx = np.meshgrid(c, c, c, indexing="ij")
    v = np.stack([cx, cy, cz], 0).astype(np.float64)
    idx = []
    for R in mats:
        g = np.einsum("ij,jdhw->idhw", R, v)
        idx.append(
            np.stack([g[2] + m, g[1] + m, g[0] + m], 0).round().astype(np.int64)
        )
    return np.stack(idx, 0)  # (24, 3, k, k, k)


def _build_program(repeat=1, mode="full"):
    import concourse.bacc as bacc
    import concourse.bass as bass
    import concourse.mybir as mybir
    import concourse.tile as tile

    f32 = mybir.dt.float32
    bf16 = mybir.dt.bfloat16
    nc = bacc.Bacc(
        "TRN2",
        target_bir_lowering=False,
        debug=False,
        enable_asserts=False,
        num_devices=N_CORES,
    )

    xs9 = nc.dram_tensor("xs9", (SLAB_D, 36, JW), bf16, kind="ExternalInput")
    w3 = nc.dram_tensor("w3", (NROW, 3 * M), bf16, kind="ExternalInput")
    bias2 = nc.dram_tensor("bias2", (MHALF, 2), f32, kind="ExternalInput")
    y = nc.dram_tensor("y", (M, DCHUNK, DHW * DHW), bf16, kind="ExternalOutput")
    xs9_ap = xs9.ap()
    y_ap = y.ap()

    with tile.TileContext(nc) as tc:
        with ExitStack() as ctx:
            wpool = ctx.enter_context(tc.tile_pool(name="wpool", bufs=1))
            epool = ctx.enter_context(tc.tile_pool(name="epool", bufs=1))
            psum = ctx.enter_context(
                tc.tile_pool(name="psum", bufs=4, space="PSUM")
            )
            stage = ctx.enter_context(tc.tile_pool(name="stage", bufs=4))

            # ---- weights + bias (tiny, once)
            w3_t = wpool.tile([NROW, 3 * M], bf16)
            nc.sync.dma_start(w3_t[:], w3.ap())
            bias_t = wpool.tile([MHALF, 2], f32)
            nc.sync.dma_start(bias_t[:], bias2.ap())

            loop_ctx = ExitStack()
            if repeat > 1:
                loop_ctx.enter_context(
                    tc.For_i(
                        0,
                        repeat,
                        1,
                        hint_engines=(
                            mybir.EngineType.PE,
                            mybir.EngineType.DVE,
                            mybir.EngineType.Activation,
                            mybir.EngineType.SP,
                            mybir.EngineType.Pool,
                        ),
                    )
                )

            # Ping-pong expanded tiles: parity t=d%2 serves plane d.
            # exp[side][t]; side 0=L (h-rows 0-31), 1=R (32-63).
            exp = [
                [
                    epool.tile([NROW, HCOLS], bf16, tag=f"exp{s}{t}",
                               name=f"exp{s}{t}")
                    for t in range(2)
                ]
                for s in range(2)
            ]

            def fresh(p, side, t):
                """DMA plane p's 36 rows from HBM into slot p%3."""
                sl = (p % 3) * 36
                nc.sync.dma_start(
                    exp[side][t][sl:sl + 36, :],
                    xs9_ap[p, :, side * HCOLS:(side + 1) * HCOLS],
                )

            def copy(p, side, t):
                """SBUF->SBUF copy of plane p's slot from the other tile."""
                sl = (p % 3) * 36
                nc.sync.dma_start(
                    exp[side][t][sl:sl + 36, :],
                    exp[side][1 - t][sl:sl + 36, :],
                )

            for s in range(2):
                for p in range(3):
                    fresh(p, s, 0)          # tile 0 <- planes 0,1,2
                copy(1, s, 1)               # tile 1 <- planes 1,2,3
                copy(2, s, 1)
                fresh(3, s, 1)

            st2 = {}  # (mh) -> 2-plane stage tile, allocated on even d
            for d in range(DCHUNK):
                r = d % 3
                t = d % 2
                # quads: (mh, side, global h-block base)
                for qi, (mh, side, hb0) in enumerate(
                    [(0, 0, 0), (1, 0, 0), (0, 1, 4), (1, 1, 4)]
                ):
                    s3 = exp[side][t]
                    if d % 2 == 0 and (mh, d) not in st2:
                        st2[(mh, d)] = stage.tile(
                            [MHALF, 16 * NCOL], bf16, tag="stage",
                            name=f"st{d}_{mh}",
                        )
                    st = st2[(mh, d - d % 2)]
                    lhsT = w3_t[:, r * M + mh * MHALF:r * M + mh * MHALF + MHALF]
                    for half in range(2):
                        ps = psum.tile(
                            [MHALF, 2 * NCOL], f32, tag="ps",
                            name=f"ps{d}_{qi}_{half}",
                        )
                        for k in range(2):
                            lb = 2 * half + k  # local h-block in this tile
                            rhs = s3[:, lb * NCOL:(lb + 1) * NCOL]
                            nc.tensor.matmul(
                                ps[:, k * NCOL:(k + 1) * NCOL],
                                lhsT,
                                rhs,
                                start=True,
                                stop=True,
                            )
                        if mode == "mm":
                            continue
                        dst = st[:, (d % 2) * 8 * NCOL
                                 + (hb0 + 2 * half) * NCOL:
                                 (d % 2) * 8 * NCOL
                                 + (hb0 + 2 * half + 2) * NCOL]
                        # DVE takes h0 halves, ACT h1 halves (staggered
                        # readiness, balanced 4/4 per plane)
                        if half == 0:
                            nc.vector.tensor_scalar_add(
                                dst, ps[:], bias_t[:, mh:mh + 1]
                            )
                        else:
                            nc.scalar.activation(
                                dst,
                                ps[:],
                                mybir.ActivationFunctionType.Identity,
                                bias=bias_t[:, mh:mh + 1],
                            )
                    # Tile t is updated for plane d+2 right after its last
                    # reader in this plane: {d+2,d+3,d+4} = keep d+2,
                    # copy d+3 from the other tile, fresh d+4 from HBM.
                    # ~2 plane-periods of slack before plane d+2 reads it.
                    if d <= DCHUNK - 3 and mode != "noexp":
                        if qi == 1:       # L side free
                            copy(d + 3, 0, t)
                            fresh(d + 4, 0, t)
                        elif qi == 3:     # R side free
                            copy(d + 3, 1, t)
                            fresh(d + 4, 1, t)
                    if d % 2 == 1 and mode not in ("mm", "noout"):
                        # 1.5MB 2-plane out-DMA via SWDGE (gpsimd): keeps
                        # the triggers off the busy evac engines entirely.
                        if qi == 2:
                            nc.gpsimd.dma_start(
                                y_ap[0:MHALF, d - 1:d + 1],
                                st2[(0, d - 1)][:],
                            )
                        elif qi == 3:
                            nc.gpsimd.dma_start(
                                y_ap[MHALF:M, d - 1:d + 1],
                                st2[(1, d - 1)][:],
                            )

            loop_ctx.close()

    nc.compile()
    return nc


def _make_runner(nc):
    """Build a reusable jitted SPMD executor (no donation so device buffers
    can be reused across timing calls). Modeled on bass2jax.run_bass_via_pjrt."""
    import jax
    import numpy as _np
    from jax.sharding import Mesh, PartitionSpec
    from jax.experimental.shard_map import shard_map

    import concourse.mybir as mybir
    from concourse import bass2jax

    bass2jax.install_neuronx_cc_hook()

    partition_name = (
        nc.partition_id_tensor.name if nc.partition_id_tensor else None
    )
    in_names, out_names, out_avals, zero_outs = [], [], [], []
    for alloc in nc.m.functions[0].allocations:
        if not isinstance(alloc, mybir.MemoryLocationSet):
            continue
        name = alloc.memorylocations[0].name
        if alloc.kind == "ExternalInput":
            if name != partition_name:
                in_names.append(name)
        elif alloc.kind == "ExternalOutput":
            shape = tuple(alloc.tensor_shape)
            dtype = mybir.dt.np(alloc.dtype)
            out_names.append(name)
            out_avals.append(jax.core.ShapedArray(shape, dtype))
            zero_outs.append(_np.zeros(shape, dtype))
    n_params = len(in_names)
    all_names = in_names + out_names
    if partition_name is not None:
        all_names = all_names + [partition_name]

    def _body(*args):
        operands = list(args)
        if partition_name is not None:
            operands.append(bass2jax.partition_id_tensor())
        outs = bass2jax._bass_exec_p.bind(
            *operands,
            out_avals=tuple(out_avals),
            in_names=tuple(all_names),
            out_names=tuple(out_names),
            lowering_input_output_aliases=(),
            sim_require_finite=True,
            sim_require_nnan=True,
            nc=nc,
        )
        return tuple(outs)

    devices = jax.devices()[:N_CORES]
    mesh = Mesh(np.asarray(devices), ("core",))
    n_args = n_params + len(out_names)
    sharded = jax.jit(
        shard_map(
            _body,
            mesh=mesh,
            in_specs=(PartitionSpec("core"),) * n_args,
            out_specs=(PartitionSpec("core"),) * len(out_names),
            check_rep=False,
        ),
        keep_unused=True,
    )

    from jax.sharding import NamedSharding

    shard = NamedSharding(mesh, PartitionSpec("core"))

    def place_inputs(in_maps):
        """Device-put per-core inputs (sharded along axis 0) + cached zero
        output buffers; returns the full arg list, all device-resident."""
        concat = [
            np.concatenate([np.asarray(m[name]) for m in in_maps], axis=0)
            for name in in_names
        ]
        placed = [jax.device_put(a, shard) for a in concat]
        zkey = "zeros_" + ",".join(
            f"{z.shape}{z.dtype}" for z in zero_outs
        )
        if zkey not in _CACHE:
            _CACHE[zkey] = [
                jax.device_put(
                    np.zeros((N_CORES * z.shape[0], *z.shape[1:]), z.dtype),
                    shard,
                )
                for z in zero_outs
            ]
        return placed + _CACHE[zkey]

    def run(args):
        return sharded(*args)

    return {
        "place_inputs": place_inputs,
        "run": run,
        "out_names": out_names,
        "out_avals": out_avals,
    }


def _get_runner():
    if "runner" not in _CACHE:
        nc = _build_program()
        _CACHE["runner"] = _make_runner(nc)
    return _CACHE["runner"]


def _get_timing_runner(repeat, mode="full"):
    key = f"runner_r{repeat}_{mode}"
    if key not in _CACHE:
        nc = _build_program(repeat=repeat, mode=mode)
        _CACHE[key] = _make_runner(nc)
    return _CACHE[key]


def _host_prep(x, weight, bias):
    import ml_dtypes

    bf16 = ml_dtypes.bfloat16
    idx = _rot_index_maps(KS)
    wr = weight[:, :, idx[:, 0], idx[:, 1], idx[:, 2]]  # (8,4,24,3,3,3)
    wr = np.transpose(wr, (2, 0, 1, 3, 4, 5)).reshape(M, CIN, KS, KS, KS)
    # wt rows ordered (kd, kh, kw, ci)
    wt = wr.transpose(2, 3, 4, 1, 0).reshape(27 * CIN, M).astype(np.float32)
    # 3 permuted lhsT variants: variant r, slot s holds taps for kd=(s-r)%3
    w3 = np.zeros((NROW, 3 * M), np.float32)
    for r in range(3):
        for s in range(3):
            kd = (s - r) % 3
            w3[36 * s:36 * s + 36, r * M:(r + 1) * M] = wt[36 * kd:36 * kd + 36]
    w3 = np.ascontiguousarray(w3).astype(bf16)
    bias192 = np.broadcast_to(bias[None, :], (N_ROT, COUT)).reshape(M)
    bias2 = np.ascontiguousarray(bias192.reshape(2, MHALF).T, dtype=np.float32)

    x_pad = np.zeros((2, CIN, PH, PH, PH), dtype=bf16)
    x_pad[:, :, 1:65, 1:65, 1:65] = x.astype(bf16)

    in_maps = []
    for core in range(N_CORES):
        n, dc = divmod(core, N_CORES // 2)
        flat3 = x_pad[n, :, DCHUNK * dc:DCHUNK * dc + SLAB_D]  # [4,18,66,66]
        xs9 = np.empty((SLAB_D, 36, JW), dtype=bf16)
        for kh in range(KS):
            for kw in range(KS):
                for ci in range(CIN):
                    xs9[:, kh * 12 + kw * 4 + ci, :] = flat3[
                        ci, :, kh:kh + DHW, kw:kw + DHW
                    ].reshape(SLAB_D, JW)
        in_maps.append({"xs9": xs9, "w3": w3, "bias2": bias2})
    return in_maps


def kernel(x, weight, bias):
    x = np.asarray(x, dtype=np.float32)
    weight = np.asarray(weight, dtype=np.float32)
    bias = np.asarray(bias, dtype=np.float32)

    runner = _get_runner()
    in_maps = _host_prep(x, weight, bias)
    args = runner["place_inputs"](in_maps)
    out = runner["run"](args)
    y8 = np.asarray(out[0]).astype(np.float32).reshape(
        N_CORES, M, DCHUNK, DHW, DHW
    )

    yfull = np.empty((2, M, DHW, DHW, DHW), dtype=np.float32)
    for core in range(N_CORES):
        n, dc = divmod(core, N_CORES // 2)
        yfull[n, :, DCHUNK * dc:DCHUNK * (dc + 1)] = y8[core]
    return yfull.reshape(2, N_ROT, COUT, DHW, DHW, DHW)


# revision 12
# speedup vs baseline: 1.0980x; 1.0980x over previous
"""Trainium2 Bass kernel for 24-rotation (octahedral) 3D conv (ConvZ3P24).

Problem: x (2,4,64,64,64) f32, weight (8,4,3,3,3), bias (8,)
  -> y (2,24,8,64,64,64):  conv3d(x, rotated_filter_bank) + bias,
  stride 1, pad 1, 24 proper octahedral rotations x 8 cout = 192 channels.

Sharding: 8 cores = batch(2) x depth-chunks(4 x 16 planes). Each core
computes all 192 channels for its 16 output planes.

Design (vs the 220us v1 baseline; measured evolution 220 -> ~150):
  - y written as bf16 (host upcasts) -> output HBM traffic halved (25.2MB).
  - input is NOT pre-im2col'd to 108 rows; host preps xs9[18, 36, 4096]:
    for each padded input plane p, the 9 (kh,kw)-shifted de-padded copies
    of the 4 cin planes (rows r=(kh,kw,ci), 64x64 contiguous per h-row).
  - device keeps ping-pong pairs of half-plane tiles exp[side][d%2]
    [108, 2048] (side 0 = h-rows 0-31, side 1 = 32-63).  Rows are 3 slots
    of 36; slot s holds plane p (p%3==s).  Tile t is rebuilt for plane
    d+2 right after plane d stops reading it (keep d+2, SBUF-copy d+3
    from the other tile, fresh-DMA d+4 from HBM) -> ~2 plane-periods of
    pipeline slack on every expansion transfer.  The kd->slot mapping is
    absorbed by 3 host-permuted lhsT variants w3[r], r=d%3 (slot s of
    variant r holds taps for kd=(s-r)%3).
  - per output plane d: 4 quads (mh, side) x 2 psum-pairs: 16 matmuls
    [108,96]^T @ [108,512] -> PSUM (contiguous rhs).  Evac in [96,1024]
    2-bank groups with fused bias add + bf16 cast; DVE owns side 0, ACT
    owns side 1, each writing its OWN 2-plane stage tile (no DVE/ACT WAW
    serialization on a shared tile).  768KB out-DMAs per (side, mh,
    2 planes) on the two HWDGE rings (SP for DVE tiles, ACT for its own),
    emitted right after the completing evac.
  - HBM/iter ~= 24.6 (out) + 5.3 (in) = ~30MB -> ~84us floor.
"""

import itertools
from contextlib import ExitStack

import numpy as np

# ---------------------------------------------------------------- constants
CIN = 4
COUT = 8
N_ROT = 24
KS = 3
DHW = 64
PH = 66            # padded plane side
JW = 4096          # xs9 row length (de-padded: 64 rows x 64 cols)
HCOLS = 2048       # half-plane tile cols (32 h-rows * 64, contiguous)
N_CORES = 8
DCHUNK = 16        # output planes per core
SLAB_D = 18        # input padded planes per core (16 + 2 halo)
M = 192            # total output channels (24 rot * 8 cout)
MHALF = 96
NROW = 108         # contraction rows = 27 taps * 4 cin
NCOL = 512         # matmul free dim (8 h-rows * 64)

_CACHE = {}


def _rot_index_maps(k):
    """Source voxel indices (d,h,w) for the 24 proper octahedral rotations."""
    m = (k - 1) // 2
    mats = []
    for perm in itertools.permutations(range(3)):
        for signs in itertools.product([1, -1], repeat=3):
            R = np.zeros((3, 3))
            for i in range(3):
                R[i, perm[i]] = signs[i]
            if np.linalg.det(R) > 0.5:
                mats.append(R)
    c = np.arange(k) - m
    cz, cy, cx = np.meshgrid(c, c, c, indexing="ij")
    v = np.stack([cx, cy, cz], 0).astype(np.float64)
    idx = []
    for R in mats:
        g = np.einsum("ij,jdhw->idhw", R, v)
        idx.append(
            np.stack([g[2] + m, g[1] + m, g[0] + m], 0).round().astype(np.int64)
        )
    return np.stack(idx, 0)  # (24, 3, k, k, k)


def _build_program(repeat=1, mode="full"):
    import concourse.bacc as bacc
    import concourse.bass as bass
    import concourse.mybir as mybir
    import concourse.tile as tile

    f32 = mybir.dt.float32
    bf16 = mybir.dt.bfloat16
    nc = bacc.Bacc(
        "TRN2",
        target_bir_lowering=False,
        debug=False,
        enable_asserts=False,
        num_devices=N_CORES,
    )

    xs9 = nc.dram_tensor("xs9", (SLAB_D, 36, JW), bf16, kind="ExternalInput")
    w3 = nc.dram_tensor("w3", (NROW, 3 * M), bf16, kind="ExternalInput")
    bias2 = nc.dram_tensor("bias2", (MHALF, 2), f32, kind="ExternalInput")
    y = nc.dram_tensor("y", (M, DCHUNK, DHW * DHW), bf16, kind="ExternalOutput")
    xs9_ap = xs9.ap()
    y_ap = y.ap()

    with tile.TileContext(nc) as tc:
        with ExitStack() as ctx:
            wpool = ctx.enter_context(tc.tile_pool(name="wpool", bufs=1))
            epool = ctx.enter_context(tc.tile_pool(name="epool", bufs=1))
            psum = ctx.enter_context(
                tc.tile_pool(name="psum", bufs=4, space="PSUM")
            )
            stage = ctx.enter_context(tc.tile_pool(name="stage", bufs=6))

            # ---- weights + bias (tiny, once)
            w3_t = wpool.tile([NROW, 3 * M], bf16)
            nc.sync.dma_start(w3_t[:], w3.ap())
            bias_t = wpool.tile([MHALF, 2], f32)
            nc.sync.dma_start(bias_t[:], bias2.ap())

            loop_ctx = ExitStack()
            if repeat > 1:
                loop_ctx.enter_context(
                    tc.For_i(
                        0,
                        repeat,
                        1,
                        hint_engines=(
                            mybir.EngineType.PE,
                            mybir.EngineType.DVE,
                            mybir.EngineType.Activation,
                            mybir.EngineType.SP,
                            mybir.EngineType.Pool,
                        ),
                    )
                )

            # Ping-pong expanded tiles: parity t=d%2 serves plane d.
            # exp[side][t]; side 0=L (h-rows 0-31), 1=R (32-63).
            exp = [
                [
                    epool.tile([NROW, HCOLS], bf16, tag=f"exp{s}{t}",
                               name=f"exp{s}{t}")
                    for t in range(2)
                ]
                for s in range(2)
            ]

            def fresh(p, side, t):
                """DMA plane p's 36 rows from HBM into slot p%3."""
                sl = (p % 3) * 36
                nc.sync.dma_start(
                    exp[side][t][sl:sl + 36, :],
                    xs9_ap[p, :, side * HCOLS:(side + 1) * HCOLS],
                )

            def copy(p, side, t):
                """SBUF->SBUF copy of plane p's slot from the other tile."""
                sl = (p % 3) * 36
                nc.sync.dma_start(
                    exp[side][t][sl:sl + 36, :],
                    exp[side][1 - t][sl:sl + 36, :],
                )

            for s in range(2):
                for p in range(3):
                    fresh(p, s, 0)          # tile 0 <- planes 0,1,2
                copy(1, s, 1)               # tile 1 <- planes 1,2,3
                copy(2, s, 1)
                fresh(3, s, 1)

            # Per-engine stage tiles (no DVE/ACT WAW on a shared tile):
            # DVE evacs the L half-plane (hb0-3), ACT the R half (hb4-7),
            # each into its own 2-plane tile [96, 2*2048].
            st2 = {}  # (side, mh, even_d) -> stage tile
            for d in range(DCHUNK):
                r = d % 3
                t = d % 2
                d0 = d - d % 2
                # quads: (mh, side, global h-block base)
                for qi, (mh, side, hb0) in enumerate(
                    [(0, 0, 0), (1, 0, 0), (0, 1, 4), (1, 1, 4)]
                ):
                    s3 = exp[side][t]
                    if d % 2 == 0 and (side, mh, d) not in st2:
                        st2[(side, mh, d)] = stage.tile(
                            [MHALF, 8 * NCOL], bf16, tag="stage",
                            name=f"st{d}_{side}_{mh}",
                        )
                    st = st2[(side, mh, d0)]
                    lhsT = w3_t[:, r * M + mh * MHALF:r * M + mh * MHALF + MHALF]
                    for half in range(2):
                        ps = psum.tile(
                            [MHALF, 2 * NCOL], f32, tag="ps",
                            name=f"ps{d}_{qi}_{half}",
                        )
                        for k in range(2):
                            lb = 2 * half + k  # local h-block in this tile
                            rhs = s3[:, lb * NCOL:(lb + 1) * NCOL]
                            nc.tensor.matmul(
                                ps[:, k * NCOL:(k + 1) * NCOL],
                                lhsT,
                                rhs,
                                start=True,
                                stop=True,
                            )
                        if mode == "mm":
                            continue
                        dst = st[:, (d % 2) * 4 * NCOL + 2 * half * NCOL:
                                 (d % 2) * 4 * NCOL + 2 * (half + 1) * NCOL]
                        if side == 0:
                            nc.vector.tensor_scalar_add(
                                dst, ps[:], bias_t[:, mh:mh + 1]
                            )
                        else:
                            nc.scalar.activation(
                                dst,
                                ps[:],
                                mybir.ActivationFunctionType.Identity,
                                bias=bias_t[:, mh:mh + 1],
                            )
                    # Tile t is updated for plane d+2 right after its last
                    # reader in this plane: {d+2,d+3,d+4} = keep d+2,
                    # copy d+3 from the other tile, fresh d+4 from HBM.
                    # ~2 plane-periods of slack before plane d+2 reads it.
                    if d <= DCHUNK - 3 and mode != "noexp":
                        if qi == 1:       # L side free
                            copy(d + 3, 0, t)
                            fresh(d + 4, 0, t)
                        elif qi == 3:     # R side free
                            copy(d + 3, 1, t)
                            fresh(d + 4, 1, t)
                    # 768KB 2-plane out-DMAs on the two HWDGE rings, each
                    # emitted right after the evac that completes its tile:
                    # DVE-written L tiles -> SP ring, ACT-written R -> ACT.
                    if d % 2 == 1 and mode not in ("mm", "noout"):
                        msl = slice(mh * MHALF, (mh + 1) * MHALF)
                        if side == 0:
                            nc.sync.dma_start(
                                y_ap[msl, d - 1:d + 1, 0:4 * NCOL],
                                st2[(0, mh, d - 1)][:],
                            )
                        else:
                            nc.scalar.dma_start(
                                y_ap[msl, d - 1:d + 1, 4 * NCOL:8 * NCOL],
                                st2[(1, mh, d - 1)][:],
                            )

            loop_ctx.close()

    nc.compile()
    return nc


def _make_runner(nc):
    """Build a reusable jitted SPMD executor (no donation so device buffers
    can be reused across timing calls). Modeled on bass2jax.run_bass_via_pjrt."""
    import jax
    import numpy as _np
    from jax.sharding import Mesh, PartitionSpec
    from jax.experimental.shard_map import shard_map

    import concourse.mybir as mybir
    from concourse import bass2jax

    bass2jax.install_neuronx_cc_hook()

    partition_name = (
        nc.partition_id_tensor.name if nc.partition_id_tensor else None
    )
    in_names, out_names, out_avals, zero_outs = [], [], [], []
    for alloc in nc.m.functions[0].allocations:
        if not isinstance(alloc, mybir.MemoryLocationSet):
            continue
        name = alloc.memorylocations[0].name
        if alloc.kind == "ExternalInput":
            if name != partition_name:
                in_names.append(name)
        elif alloc.kind == "ExternalOutput":
            shape = tuple(alloc.tensor_shape)
            dtype = mybir.dt.np(alloc.dtype)
            out_names.append(name)
            out_avals.append(jax.core.ShapedArray(shape, dtype))
            zero_outs.append(_np.zeros(shape, dtype))
    n_params = len(in_names)
    all_names = in_names + out_names
    if partition_name is not None:
        all_names = all_names + [partition_name]

    def _body(*args):
        operands = list(args)
        if partition_name is not None:
            operands.append(bass2jax.partition_id_tensor())
        outs = bass2jax._bass_exec_p.bind(
            *operands,
            out_avals=tuple(out_avals),
            in_names=tuple(all_names),
            out_names=tuple(out_names),
            lowering_input_output_aliases=(),
            sim_require_finite=True,
            sim_require_nnan=True,
            nc=nc,
        )
        return tuple(outs)

    devices = jax.devices()[:N_CORES]
    mesh = Mesh(np.asarray(devices), ("core",))
    n_args = n_params + len(out_names)
    sharded = jax.jit(
        shard_map(
            _body,
            mesh=mesh,
            in_specs=(PartitionSpec("core"),) * n_args,
            out_specs=(PartitionSpec("core"),) * len(out_names),
            check_rep=False,
        ),
        keep_unused=True,
    )

    from jax.sharding import NamedSharding

    shard = NamedSharding(mesh, PartitionSpec("core"))

    def place_inputs(in_maps):
        """Device-put per-core inputs (sharded along axis 0) + cached zero
        output buffers; returns the full arg list, all device-resident."""
        concat = [
            np.concatenate([np.asarray(m[name]) for m in in_maps], axis=0)
            for name in in_names
        ]
        placed = [jax.device_put(a, shard) for a in concat]
        zkey = "zeros_" + ",".join(
            f"{z.shape}{z.dtype}" for z in zero_outs
        )
        if zkey not in _CACHE:
            _CACHE[zkey] = [
                jax.device_put(
                    np.zeros((N_CORES * z.shape[0], *z.shape[1:]), z.dtype),
                    shard,
                )
                for z in zero_outs
            ]
        return placed + _CACHE[zkey]

    def run(args):
        return sharded(*args)

    return {
        "place_inputs": place_inputs,
        "run": run,
        "out_names": out_names,
        "out_avals": out_avals,
    }


def _get_runner():
    if "runner" not in _CACHE:
        nc = _build_program()
        _CACHE["runner"] = _make_runner(nc)
    return _CACHE["runner"]


def _get_timing_runner(repeat, mode="full"):
    key = f"runner_r{repeat}_{mode}"
    if key not in _CACHE:
        nc = _build_program(repeat=repeat, mode=mode)
        _CACHE[key] = _make_runner(nc)
    return _CACHE[key]


def _host_prep(x, weight, bias):
    import ml_dtypes

    bf16 = ml_dtypes.bfloat16
    idx = _rot_index_maps(KS)
    wr = weight[:, :, idx[:, 0], idx[:, 1], idx[:, 2]]  # (8,4,24,3,3,3)
    wr = np.transpose(wr, (2, 0, 1, 3, 4, 5)).reshape(M, CIN, KS, KS, KS)
    # wt rows ordered (kd, kh, kw, ci)
    wt = wr.transpose(2, 3, 4, 1, 0).reshape(27 * CIN, M).astype(np.float32)
    # 3 permuted lhsT variants: variant r, slot s holds taps for kd=(s-r)%3
    w3 = np.zeros((NROW, 3 * M), np.float32)
    for r in range(3):
        for s in range(3):
            kd = (s - r) % 3
            w3[36 * s:36 * s + 36, r * M:(r + 1) * M] = wt[36 * kd:36 * kd + 36]
    w3 = np.ascontiguousarray(w3).astype(bf16)
    bias192 = np.broadcast_to(bias[None, :], (N_ROT, COUT)).reshape(M)
    bias2 = np.ascontiguousarray(bias192.reshape(2, MHALF).T, dtype=np.float32)

    x_pad = np.zeros((2, CIN, PH, PH, PH), dtype=bf16)
    x_pad[:, :, 1:65, 1:65, 1:65] = x.astype(bf16)

    in_maps = []
    for core in range(N_CORES):
        n, dc = divmod(core, N_CORES // 2)
        flat3 = x_pad[n, :, DCHUNK * dc:DCHUNK * dc + SLAB_D]  # [4,18,66,66]
        xs9 = np.empty((SLAB_D, 36, JW), dtype=bf16)
        for kh in range(KS):
            for kw in range(KS):
                for ci in range(CIN):
                    xs9[:, kh * 12 + kw * 4 + ci, :] = flat3[
                        ci, :, kh:kh + DHW, kw:kw + DHW
                    ].reshape(SLAB_D, JW)
        in_maps.append({"xs9": xs9, "w3": w3, "bias2": bias2})
    return in_maps


def kernel(x, weight, bias):
    x = np.asarray(x, dtype=np.float32)
    weight = np.asarray(weight, dtype=np.float32)
    bias = np.asarray(bias, dtype=np.float32)

    runner = _get_runner()
    in_maps = _host_prep(x, weight, bias)
    args = runner["place_inputs"](in_maps)
    out = runner["run"](args)
    y8 = np.asarray(out[0]).astype(np.float32).reshape(
        N_CORES, M, DCHUNK, DHW, DHW
    )

    yfull = np.empty((2, M, DHW, DHW, DHW), dtype=np.float32)
    for core in range(N_CORES):
        n, dc = divmod(core, N_CORES // 2)
        yfull[n, :, DCHUNK * dc:DCHUNK * (dc + 1)] = y8[core]
    return yfull.reshape(2, N_ROT, COUT, DHW, DHW, DHW)
